# revision 1
# baseline (speedup 1.0000x reference)
"""Chunked sliding-window attention (window=256) fused kernel for Trainium2.

Reference computation (B=2, S=8192, Dm=512, H=8, hd=64, W=256):
    q/k/v = x @ W{q,k,v}.T ; per-head sliding-window attention; out = attn @ Wo.T

Sharding: sequence-parallel over 8 cores: core c handles batch b = c//4,
tokens [(c%4)*2048, (c%4+1)*2048), plus a 256-token halo of k/v context.
Every core runs the same Bass program (SPMD); the halo of chunk-0 cores is
zero-filled and masked out via a per-core block-0 mask (data, not program).

Per-core structure, per 256-token block n in {-1..7} (n=-1: kv-only halo):
  xt   [128, 4, 256] bf16 : x.T block (d-chunks on partitions)
  kf/qf [128, 4, 256] bf16: k.T/q.T (features on partitions; head h = rows
                            (h%2)*64..+64 of f-chunk h//2); single psum copy
  v_il [128, 8, 65] x2 bf16: v token-major per-head groups + ones column
                            (col 64) that accumulates the softmax sums
  scores (per head pair) psum [128, 8, 256] f32: keys on partitions, free =
      (head, key-chunk, query); odd head's matmuls use tile_position=(64,0)
      so both heads' K=64 matmuls share the PE array concurrently
  PT = exp(scores/8)*mask, bf16 [128, 2048] (one ACT op + one DVE mult/pair)
  PV per (t-half, head-quad): psum [128, 4, 128]: out[t, dd] + l[t];
      batched reciprocal + broadcast multiply normalize -> a_raw [128, 512]
  PE-transpose a_raw -> aT [f, t]; final matmul vs Wo.T -> out [256, 512] f32
"""

import numpy as np
import ml_dtypes

import concourse.bass as bass
import concourse.mybir as mybir
import concourse.tile as tile
from concourse.masks import make_identity
from concourse.bass_utils import run_bass_kernel_spmd

BF16 = mybir.dt.bfloat16
F32 = mybir.dt.float32

B, S, DM = 2, 8192, 512
H, HD, W = 8, 64, 256
NCORES = 8
CHUNK = (B * S) // NCORES          # 2048 tokens per core
NBLK = CHUNK // W                  # 8 query blocks per core
SLOC = W + CHUNK                   # 2304 tokens incl. halo


def _build_masks():
    """Pair mask [128, 2, 4, 256]: mask[p, i, ck, qi] (same for both heads i).

    Valid iff qi+1 <= kidx <= qi+256, kidx = ck*128+p over prev||cur blocks.
    mask0 additionally requires kidx >= 256 (chunk-0 cores' first block).
    """
    # 128-query sub-blocks: keys are the 3 aligned 128-chunks ending at the
    # sub-block end; kr = ckj*128+ki in [0, 384); valid iff qi+1 <= kr <= qi+256
    kr = (np.arange(3)[:, None] * 128 + np.arange(128)[None, :]).T  # [128, 3]
    qi = np.arange(128)
    valid = (kr[:, :, None] >= qi[None, None, :] + 1) & (
        kr[:, :, None] <= qi[None, None, :] + 256
    )  # [128, 3, 128]

    def pack(v_th0, v_th1):
        # layout [p, th, i(head-in-pair), ckj, qi] -> [128, 2, 768] contiguous
        m = np.stack([v_th0, v_th1], axis=1)[:, :, None]          # [p, th, 1, 3, qi]
        m = np.broadcast_to(m, (128, 2, 2, 3, 128))
        return np.ascontiguousarray(m).reshape(128, 2 * 2 * 3 * 128).astype(ml_dtypes.bfloat16)

    v = valid.astype(np.float32)
    v0_th0 = v.copy(); v0_th0[:, 0:2, :] = 0.0   # block 0, sub 0: both prev chunks invalid
    v0_th1 = v.copy(); v0_th1[:, 0:1, :] = 0.0   # block 0, sub 1: one prev chunk invalid
    # compact mask for n>=1: only ckj 0 and 2 (ckj 1 always fully valid);
    # layout [p, i, cksel(2), qi] -> [128, 512], same for both sub-blocks
    mc = np.stack([v[:, 0, :], v[:, 2, :]], axis=1)          # [p, 2, qi]
    mc = np.broadcast_to(mc[:, None], (128, 2, 2, 128))
    mask_c = np.ascontiguousarray(mc).reshape(128, 512).astype(ml_dtypes.bfloat16)
    return pack(v, v), pack(v0_th0, v0_th1), mask_c


def _split_waits(nc, max_waits=1):
    """Walrus here rejects >1 sync wait per instruction; hoist extras onto NoOps."""
    for fn in nc.m.functions:
        for bb in fn.blocks:
            newlist = []
            changed = False
            for inst in bb.instructions:
                si = inst.sync_info
                if si is not None and si.on_wait is not None and len(si.on_wait) > max_waits:
                    waits = list(si.on_wait)
                    head, tail = waits[:-max_waits], waits[-max_waits:]
                    for ci, i0 in enumerate(range(0, len(head), max_waits)):
                        nop = mybir.InstNoOp(name=f"{inst.name}-wsplit{ci}", ins=[], outs=[])
                        nop.engine = inst.engine
                        nop.sync_info = mybir.SyncInfo(on_wait=head[i0:i0 + max_waits], on_update=[])
                        newlist.append(nop)
                    inst.sync_info = mybir.SyncInfo(on_wait=tail, on_update=si.on_update)
                    changed = True
                newlist.append(inst)
            if changed:
                bb.instructions = newlist


def build_nc():
    nc = bass.Bass(target_bir_lowering=False)

    xT = nc.dram_tensor("xT", [DM, SLOC], BF16, kind="ExternalInput")
    Wall = nc.dram_tensor("Wall", [DM, 4 * DM], BF16, kind="ExternalInput")
    Mall = nc.dram_tensor("Mall", [128, 2 * 768 + 2 * 768 + 512], BF16, kind="ExternalInput")
    out = nc.dram_tensor("out", [CHUNK, DM], F32, kind="ExternalOutput")

    with tile.TileContext(nc) as tc:
        with (
            tc.tile_pool(name="const", bufs=1) as const,
            tc.tile_pool(name="xt", bufs=5) as xt_pool,
            tc.tile_pool(name="qf", bufs=3) as qf_pool,
            tc.tile_pool(name="kf", bufs=4) as kf_pool,
            tc.tile_pool(name="vil", bufs=6) as vil_pool,
            tc.tile_pool(name="pt", bufs=10) as pt_pool,
            tc.tile_pool(name="rc", bufs=10) as rc_pool,
            tc.tile_pool(name="araw", bufs=4) as araw_pool,
            tc.tile_pool(name="at", bufs=8) as at_pool,
            tc.tile_pool(name="fin", bufs=4) as fin_pool,
            tc.tile_pool(name="proj_ps", bufs=2, space="PSUM") as proj_ps,
            tc.tile_pool(name="sc_ps", bufs=2, space="PSUM") as sc_ps,
            tc.tile_pool(name="pv_ps", bufs=2, space="PSUM") as pv_ps,
        ):
            # ---- constants (single DMA each for weights and masks) ----
            wall = const.tile([128, 4, 4 * DM], BF16)
            nc.sync.dma_start(wall[:], Wall[:].rearrange("(c p) f -> p c f", p=128))
            wk = wall[:, :, 0 * DM:1 * DM]
            wv = wall[:, :, 1 * DM:2 * DM]
            wq = wall[:, :, 2 * DM:3 * DM]
            wo = wall[:, :, 3 * DM:4 * DM]
            mall = const.tile([128, 2 * 768 + 2 * 768 + 512], BF16)
            nc.scalar.dma_start(mall[:], Mall[:])
            maskN = mall[:, 0:1536].rearrange("p (c r) -> p c r", c=2)
            mask0 = mall[:, 1536:3072].rearrange("p (c r) -> p c r", c=2)
            maskC = mall[:, 3072:3584].rearrange("p (i c t) -> p i c t", i=2, c=2)
            ident = const.tile([128, 128], BF16)
            make_identity(nc, ident[:])

            k_prev = None              # kf tile of previous block
            v_prev = [None, None]      # v_il tiles (2 t-halves) of previous block

            for n in range(-1, NBLK):
                col0 = (n + 1) * W
                xt = xt_pool.tile([128, 4, W], BF16, tag="xt")
                nc.sync.dma_start(
                    xt[:], xT[:, col0:col0 + W].rearrange("(c p) t -> p c t", p=128)
                )

                # -- kT projection [f 128, (fc, t)] + single eviction copy --
                k_cur = kf_pool.tile([128, 4, W], BF16, tag="kf")
                for half in range(2):
                    kps = proj_ps.tile([128, 2, W], F32, tag="proj")
                    for fc2 in range(2):
                        fc = 2 * half + fc2
                        for dc in range(4):
                            nc.tensor.matmul(
                                kps[:, fc2, :],
                                wk[:, dc, fc * 128:(fc + 1) * 128],
                                xt[:, dc, :],
                                start=(dc == 0), stop=(dc == 3),
                            )
                    nc.vector.tensor_copy(k_cur[:, 2 * half:2 * half + 2, :], kps[:])

                # -- v projection -> interleaved v_il tiles --
                v_cur = []
                for th in range(2):
                    vps = proj_ps.tile([128, DM], F32, tag="proj")
                    for dc in range(4):
                        nc.tensor.matmul(
                            vps[:],
                            xt[:, dc, th * 128:(th + 1) * 128],
                            wv[:, dc, :],
                            start=(dc == 0), stop=(dc == 3),
                        )
                    vt = vil_pool.tile([128, H, 65], BF16, tag="vil")
                    nc.vector.memset(vt[:, :, 64:65], 1.0)
                    nc.scalar.copy(
                        vt[:, :, 0:64],
                        vps[:].rearrange("p (h x) -> p h x", h=H),
                    )
                    v_cur.append(vt)

                if n >= 0:
                    # -- qT projection --
                    q_cur = qf_pool.tile([128, 4, W], BF16, tag="qf")
                    for half in range(2):
                        qps = proj_ps.tile([128, 2, W], F32, tag="proj")
                        for fc2 in range(2):
                            fc = 2 * half + fc2
                            for dc in range(4):
                                nc.tensor.matmul(
                                    qps[:, fc2, :],
                                    wq[:, dc, fc * 128:(fc + 1) * 128],
                                    xt[:, dc, :],
                                    start=(dc == 0), stop=(dc == 3),
                                )
                        nc.vector.tensor_copy(q_cur[:, 2 * half:2 * half + 2, :], qps[:])

                    mask = mask0 if n == 0 else maskN
                    a_raw0 = araw_pool.tile([128, DM], BF16, tag="araw")
                    a_raw1 = araw_pool.tile([128, DM], BF16, tag="araw")
                    a_raw = [a_raw0, a_raw1]
                    pt_sub = {}

                    def scores(p, ths=(0, 1)):
                        for th in ths:         # 128-query sub-block
                            scps = sc_ps.tile([128, 2, 4, 128], F32, tag="sc")
                            for i in range(2):
                                for ckj in range(3):
                                    cid = th + ckj   # absolute 128-chunk id over prev||cur
                                    ksrc = k_prev if cid < 2 else k_cur
                                    nc.tensor.matmul(
                                        scps[:, i, ckj, :],
                                        ksrc[i * 64:i * 64 + 64, p, (cid % 2) * 128:(cid % 2) * 128 + 128],
                                        q_cur[i * 64:i * 64 + 64, p, th * 128:(th + 1) * 128],
                                        start=True, stop=True,
                                        tile_position=(i * 64, 0),
                                    )
                            ptt = pt_pool.tile([128, 2, 3 * 128], BF16, tag="pt")
                            nc.scalar.activation(
                                ptt[:],
                                scps[:, :, 0:3, :].rearrange("p i c t -> p i (c t)"),
                                mybir.ActivationFunctionType.Exp, scale=0.125,
                            )
                            if n == 0:
                                nc.vector.tensor_mul(
                                    ptt[:].rearrange("p i r -> p (i r)"),
                                    ptt[:].rearrange("p i r -> p (i r)"),
                                    mask[:, th, :],
                                )
                            else:
                                pv4 = ptt[:].rearrange("p i (c t) -> p i c t", c=3)
                                nc.vector.tensor_tensor(
                                    pv4[:, :, 0:3:2, :], pv4[:, :, 0:3:2, :], maskC,
                                    mybir.AluOpType.mult,
                                )
                            pt_sub[(p, th)] = ptt[:].rearrange("p i (c t) -> p i c t", c=3)

                    def pv(p, ths=(0, 1)):
                        for th in ths:
                            ptv = pt_sub[(p, th)]
                            pvp = pv_ps.tile([128, 2, 128], F32, tag="pv")
                            for i in range(2):
                                h = 2 * p + i
                                for ckj in range(3):
                                    cid = th + ckj
                                    vsrc = v_prev[cid % 2] if cid < 2 else v_cur[cid % 2]
                                    nc.tensor.matmul(
                                        pvp[:, i, 0:65],
                                        ptv[:, i, ckj, :],
                                        vsrc[:, h, :],
                                        start=(ckj == 0), stop=(ckj == 2),
                                    )
                            rec = rc_pool.tile([128, 2], F32, tag="rc")
                            nc.vector.reciprocal(rec[:], pvp[:, :, 64:65].rearrange("p j o -> p (j o)"))
                            nc.vector.tensor_tensor(
                                a_raw[th][:, p * 128:(p + 1) * 128].rearrange("p2 (j x) -> p2 j x", j=2),
                                pvp[:, :, 0:64],
                                rec[:, :, None].broadcast_to([128, 2, 64]),
                                mybir.AluOpType.mult,
                            )

                    scores(0, (0,))
                    scores(0, (1,))
                    pv(0, (0,))
                    for p in range(1, 4):
                        scores(p, (0,))
                        pv(p - 1, (1,))
                        scores(p, (1,))
                        pv(p, (0,))
                    pv(3, (1,))

                    # -- transpose a_raw -> aT, interleaved with final accumulation --
                    at_tiles = []
                    for fc in range(4):
                        tp = sc_ps.tile([128, 2, 128], BF16, tag="sc")
                        for th in range(2):
                            nc.tensor.transpose(
                                tp[:, th, :],
                                a_raw[th][:, fc * 128:(fc + 1) * 128],
                                ident[:],
                            )
                        att = at_pool.tile([128, 2 * 128], BF16, tag="at")
                        nc.vector.tensor_copy(att[:], tp[:].rearrange("p c t -> p (c t)"))
                        at_tiles.append(att)
                    for th in range(2):
                        fps = pv_ps.tile([128, DM], F32, tag="pv")
                        for fc in range(4):
                            nc.tensor.matmul(
                                fps[:],
                                at_tiles[fc][:, th * 128:(th + 1) * 128],
                                wo[:, fc, :],
                                start=(fc == 0), stop=(fc == 3),
                            )
                        fin = fin_pool.tile([128, DM], F32, tag="fin")
                        nc.scalar.copy(fin[:], fps[:])
                        nc.sync.dma_start(
                            out[n * W + th * 128:n * W + th * 128 + 128, :], fin[:]
                        )

                k_prev = k_cur
                v_prev = v_cur

    _split_waits(nc)
    return nc


_NC_CACHE = None


def kernel(x, Wq, Wk, Wv, Wo):
    global _NC_CACHE
    x = np.asarray(x, np.float32)
    mask_n, mask_0, mask_c = _build_masks()

    wall = np.concatenate([np.asarray(w, np.float32).T for w in (Wk, Wv, Wq, Wo)], axis=1)
    wall = np.ascontiguousarray(wall).astype(ml_dtypes.bfloat16)

    in_maps = []
    for c in range(NCORES):
        b, ch = divmod(c, NCORES // B)
        t0 = ch * CHUNK
        xs = np.zeros((SLOC, DM), np.float32)
        lo = max(t0 - W, 0)
        xs[W - (t0 - lo):] = x[b, lo:t0 + CHUNK]
        xTc = np.ascontiguousarray(xs.T).astype(ml_dtypes.bfloat16)
        mall = np.concatenate(
            [mask_n, mask_0 if ch == 0 else mask_n, mask_c], axis=1
        ).astype(ml_dtypes.bfloat16)
        in_maps.append({"xT": xTc, "Wall": wall, "Mall": np.ascontiguousarray(mall)})

    if _NC_CACHE is None:
        _NC_CACHE = build_nc()
    res = run_bass_kernel_spmd(_NC_CACHE, in_maps, core_ids=list(range(NCORES)))
    outs = [res.results[c]["out"] for c in range(NCORES)]
    full = np.stack(outs).reshape(B, S, DM)
    return full.astype(np.float32)

